# revision 77
# baseline (speedup 1.0000x reference)
"""Causal self-attention (RoPE) Trainium2 kernel, 8-way tensor-parallel.

Sharding (Megatron-style, zero-cost input distribution since every core
receives the full inputs): core c owns global heads {2c, 2c+1}.

All large matmuls run as fp8e4m3 DoubleRow pairs (2 contraction k-tiles
per instruction, 0.5 cycles/row): operands are split host-side into an
e4m3 hi part + e4m3 residual lo, and each product is computed as three
term groups (hi*hi + lo*hi + hi*lo), which restores ~fp16 accuracy at
3/4 of the fp16 PE cost. Weights are pre-scaled by SW=128 host-side so
their residuals stay out of the e4m3 subnormal range; the scale is
divided back out in the epilogues (QKV: ACT scale; proj: host divide).

Per core:
  1. qkv^T = W_slice^T @ x^T (fp8 DR 3-term), one merged multi-tile DMA
     per 512-token chunk; per-chunk epilogue: ACT bias+1/SW for q/k,
     RoPE on q/k (pair-swap DMA + 3 DVE ops per (head, q/k)). v strips
     are computed directly in [token, d] layout (lhsT = x pair, rhs = w
     pair) -- no PE transpose; v bias is folded into b_proj host-side.
  2. Per (head, batch): causal flash attention in the TRANSPOSED (S^T)
     orientation: fp16 score tiles [k, q] (trimmed to the causal range)
     come straight off the PE with k on partitions, the triangular mask
     lands via a second PE matmul (ident^T @ cm) into the same psum,
     exp (ACT) writes P^T into SBUF, and fp16 PV matmuls stream it. The
     softmax denominator needs NO PE work: a DVE running sum of the exp
     tiles + one gpsimd partition-all-reduce; DVE then scales y^T by
     the reciprocal and emits fp8 hi/lo halves (copy on Pool, sub on
     DVE) packed side by side in one agin buffer.
  3. AllGather (fp8 hi|lo, one per head/batch/quarter -- T/512-token
     granularity) so projection starts while later attention runs
  4. Output projection vs the core's 256-column slice of W_proj (fp8 DR
     3-term over head-pair k-tiles) in two parity halves (even heads ->
     fp16 partials in SBUF, odd heads + partial + bias -> fp16 out,
     host-upcast), interleaved into the attention stream as PE filler;
     mid-stream groups cycle 3 psum banks, tail groups 4.

Attention for batch 0 is interleaved into QKV chunks 4-7 (its qkv data
is complete after chunk 3), sharing one rotating PSUM pool between QKV
strip accumulators and attention score tiles to stay within 8 banks.

Host side shards/splits weights into fp8 hi/lo, builds RoPE/mask
tables, and concatenates + rescales the 8 column slices into the final
[B, T, C] float32 output.
"""

import functools
import numpy as np

import concourse.bass as bass
import concourse.bass_isa as bass_isa
import concourse.mybir as mybir
import concourse.tile as tile
from concourse import bacc
from concourse.bass_utils import run_bass_kernel_spmd
from concourse.masks import make_identity
from concourse.tile import add_dep_helper

F32 = mybir.dt.float32
F16 = mybir.dt.float16
F8 = mybir.dt.float8e4
DR = mybir.MatmulPerfMode.DoubleRow
SW = 128.0  # host-side W_attn pre-scale; epilogue divides it back out

N_CORES = 8
C = 2048           # model dim
H = 16             # total heads
HD = 128           # head dim
HL = 2             # heads per core
OC = C // N_CORES  # output cols per core (256)
SCALE = 1.0 / float(np.sqrt(HD))
MASK_VAL = -900.0  # additive pre-scale mask; exp arg ~ -80 -> underflows to 0


def build(B=2, T=2048, collective=True, n_cores=N_CORES):
    """Build the SPMD Bass program (identical on every core)."""
    BT = B * T
    NSTR = 3 * HL                  # qkv strips of 128 cols
    NCT = C // 128                 # contraction tiles
    NTCH = BT // 512               # token chunks for qkv
    NQC = T // 512                 # q chunks per (b, h)
    NTT = T // 128                 # token tiles per batch

    nc = bacc.Bacc(None, target_bir_lowering=False)
    xT_hi = nc.dram_tensor("xT_hi", [C, BT], F8, kind="ExternalInput")
    xT_lo = nc.dram_tensor("xT_lo", [C, BT], F8, kind="ExternalInput")
    wq_hi = nc.dram_tensor("wq_hi", [C, NSTR * 128], F8, kind="ExternalInput")
    wq_lo = nc.dram_tensor("wq_lo", [C, NSTR * 128], F8, kind="ExternalInput")
    bqkv = nc.dram_tensor("bqkv", [NSTR * 128, 1], F32, kind="ExternalInput")
    ctil = nc.dram_tensor("ctil", [128, T], F16, kind="ExternalInput")
    stil = nc.dram_tensor("stil", [128, T], F16, kind="ExternalInput")
    wp_hi = nc.dram_tensor("wp_hi", [C, OC], F8, kind="ExternalInput")
    wp_lo = nc.dram_tensor("wp_lo", [C, OC], F8, kind="ExternalInput")
    bpb = nc.dram_tensor("bpb", [128, OC], F32, kind="ExternalInput")
    cmask = nc.dram_tensor("cmask", [128, 128], F16, kind="ExternalInput")
    out = nc.dram_tensor("out", [BT, OC], F16, kind="ExternalOutput")

    with tile.TileContext(nc) as tc:
        with (
            tc.tile_pool(name="big", bufs=1) as big,
            tc.tile_pool(name="dram", bufs=1, space="DRAM") as dram,
            tc.tile_pool(name="pt", bufs=2) as pt_pool,
            tc.tile_pool(name="stat", bufs=2) as stat_pool,
            tc.tile_pool(name="yts", bufs=3) as yts_pool,
            # one rotating psum pool shared by QKV strip accumulators and
            # attention score tiles (same [128, 512] f32 shape/tag)
            tc.tile_pool(name="mm_ps", bufs=3, space="PSUM") as mm_ps,
            tc.tile_pool(name="y_ps", bufs=2, space="PSUM") as y_ps,
            tc.tile_pool(name="l_ps", bufs=1, space="PSUM") as l_ps,
        ):
            # ---- persistent SBUF tensors ----
            qr = big.tile([128, HL * BT], F16, tag="qr")
            kr = big.tile([128, HL * BT], F16, tag="kr")
            v_sb = big.tile([128, HL * BT], F16, tag="v_sb")
            ct_sb = big.tile([128, T], F16, tag="ct")
            st_sb = big.tile([128, T], F16, tag="st")
            ident = big.tile([128, 128], F16, tag="ident")
            cm_sb = big.tile([128, 128], F16, tag="cm")
            bq_sb = big.tile([128, NSTR], F32, tag="bq")
            bp_sb = big.tile([128, OC], F32, tag="bp")

            # DRAM bounce buffers: one AllGather per (local head, batch,
            # column half) so projection starts on the first half while the
            # second is still on the wire. y^T ships as fp8 hi+lo halves
            # packed side by side (hi cols [0,hw_), lo cols [hw_,2hw_)) so
            # the byte count matches the old fp16 buffers.
            split_ag = T >= 1024
            nh = 4 if T >= 2048 else (2 if split_ag else 1)
            hw_ = T // nh
            hqc = max(1, NQC // nh)
            agin = {}
            agout = {}
            for j in range(HL):
                for b in range(B):
                    for h in range(nh):
                        agin[(j, b, h)] = dram.tile([128, 2 * hw_], F8,
                                                    name=f"agin{j}_{b}_{h}")
                        agout[(j, b, h)] = dram.tile([n_cores * 128, 2 * hw_],
                                                     F8,
                                                     name=f"agout{j}_{b}_{h}")

            partial = {}
            cc_insts = {}
            agin_dmas = {}

            def emit_ag(j, b, h):
                if collective:
                    cc = nc.gpsimd.collective_compute(
                        "AllGather",
                        mybir.AluOpType.bypass,
                        replica_groups=[list(range(n_cores))],
                        ins=[agin[(j, b, h)].opt()],
                        outs=[agout[(j, b, h)].opt()],
                    )
                else:
                    cc = nc.sync.dma_start(agout[(j, b, h)][0:128, :],
                                           agin[(j, b, h)][:, :])
                cci = cc.ins if hasattr(cc, "ins") else cc
                for d in agin_dmas[(j, b)][h * hqc:(h + 1) * hqc]:
                    di = d.ins if hasattr(d, "ins") else d
                    add_dep_helper(cci, di,
                                   reason="collective reads agin after y")
                cc_insts.setdefault((j, b), {})[h] = cci

            def attn_qc(j, b, qc, filler=None, last=False):
                """One flash unit in S^T orientation: score tiles [k, q] ->
                exp -> PV matmul, 2-deep software pipeline on the score
                psum. The softmax denominator is a DVE running sum of the
                exp tiles + one gpsimd partition-all-reduce (no PE work).
                `filler` emits independent PE work after the first two score
                matmuls to cover the exp latency."""
                base = j * BT + b * T
                K = 4 * qc + 4
                ptall = pt_pool.tile([128, 8192], F16, tag="ptall",
                                     name="ptall")
                psy = y_ps.tile([128, 512], F32, tag="psy", name="psy")
                ssum = stat_pool.tile([128, 512], F16, tag="ssum",
                                      name="ssum")
                q_ap = qr[:, base + qc * 512: base + (qc + 1) * 512]

                def consume(kt):
                    qs = max(0, kt - 4 * qc) * 128
                    pt_ap = ptall[:, kt * 512 + qs:(kt + 1) * 512]
                    nc.tensor.matmul(
                        psy[:, qs:512],
                        v_sb[:, base + kt * 128: base + (kt + 1) * 128],
                        pt_ap, start=(kt == 0), stop=(kt == K - 1))

                for kt in range(K):
                    qs = max(0, kt - 4 * qc) * 128
                    m = kt - 4 * qc
                    sps = mm_ps.tile([128, 512], F32, tag="ps512", name="sps")
                    nc.tensor.matmul(
                        sps[:, qs:512],
                        kr[:, base + kt * 128: base + (kt + 1) * 128],
                        q_ap[:, qs:512], start=True, stop=(m < 0))
                    if m >= 0:
                        # diagonal-band tile: transposed triangular mask,
                        # added on the PE (ident^T @ cm = cm) to keep DVE
                        # off the score->exp critical path
                        nc.tensor.matmul(
                            sps[:, qs:qs + 128], ident[:], cm_sb[:],
                            start=False, stop=True)
                    pt_ap = ptall[:, kt * 512 + qs:(kt + 1) * 512]
                    nc.scalar.activation(
                        pt_ap, sps[:, qs:512],
                        mybir.ActivationFunctionType.Exp, scale=SCALE)
                    if kt == 0:
                        nc.vector.tensor_copy(ssum[:], pt_ap)
                    else:
                        nc.vector.tensor_add(ssum[:, qs:512],
                                             ssum[:, qs:512], pt_ap)
                    if filler is not None and kt % 2 == 1 and \
                            kt // 2 < len(filler):
                        filler[kt // 2]()
                    if kt >= 3:
                        consume(kt - 3)
                for kt_t in range(K - 3, K):
                    consume(kt_t)

                sall = stat_pool.tile([128, 512], F32, tag="sall",
                                      name="sall")
                nc.gpsimd.partition_all_reduce(sall[:], ssum[:], 128,
                                               bass_isa.ReduceOp.add)
                recb = stat_pool.tile([128, 512], F32, tag="recb",
                                      name="recb")
                nc.vector.reciprocal(recb[:], sall[:])
                yt = yts_pool.tile([128, 512], F16, tag="yt", name="yt")
                nc.vector.tensor_mul(yt[:], psy[:], recb[:])
                # fp8 hi + residual lo of y^T, shipped as one packed DMA
                yt8 = yts_pool.tile([128, 1024], F8, tag="yt8", name="yt8")
                if last:
                    nc.vector.tensor_copy(yt8[:, 0:512], yt[:])
                else:
                    nc.gpsimd.tensor_copy(yt8[:, 0:512], yt[:])
                nc.vector.tensor_sub(yt8[:, 512:1024], yt[:], yt8[:, 0:512])
                q0 = (qc % hqc) * 512
                d = nc.gpsimd.dma_start(
                    agin[(j, b, qc // hqc)][:].rearrange(
                        "p (h t) -> p h t", t=hw_)[:, :, q0:q0 + 512],
                    yt8[:].rearrange("p (h t) -> p h t", t=512))
                agin_dmas.setdefault((j, b), []).append(d)
                if (qc + 1) % hqc == 0:
                    emit_ag(j, b, qc // hqc)

            # W_proj fp8 tiles live in the persistent pool; their loads are
            # emitted late in Phase A so Phase B starts compute-ready
            wp_hi_all = big.tile([128, H * OC], F8, tag="wpall_h",
                                 name="wpall_h")
            wp_lo_all = big.tile([128, H * OC], F8, tag="wpall_l",
                                 name="wpall_l")

            def load_wp():
                for wp_all, wp_src in ((wp_hi_all, wp_hi), (wp_lo_all, wp_lo)):
                    for half in range(2):
                        nc.sync.dma_start(
                            wp_all[:, half * 8 * OC:(half + 1) * 8 * OC]
                            .rearrange("p (c t) -> p c t", t=OC),
                            wp_src[half * 8 * 128:(half + 1) * 8 * 128, :]
                            .rearrange("(c p) t -> p c t", p=128))

            # ================= Phase A: QKV + RoPE + v-transpose ============
            with (
                tc.tile_pool(name="wq", bufs=1) as wq_pool,
                tc.tile_pool(name="xt", bufs=2) as xt_pool,
                tc.tile_pool(name="rope", bufs=2) as rope_pool,
                tc.tile_pool(name="stage", bufs=2) as stage_pool,
                tc.tile_pool(name="vt_ps", bufs=2, space="PSUM") as vt_ps,
            ):
                # Merged multi-tile DMAs: one SBUF tile holds all NCT 128-row
                # blocks side by side; a single DMA with a 3-d access pattern
                # fills it. QKV runs as fp8 DoubleRow matmuls: x and W are
                # split host-side into e4m3 hi + residual lo; three term
                # groups (hi*hi, lo*hi, hi*lo) accumulate in one psum, each
                # as NCT/2 two-ktile DR matmuls.
                w_hi_all = wq_pool.tile([128, NCT * NSTR * 128], F8,
                                        tag="wall_h", name="wall_h")
                w_lo_all = wq_pool.tile([128, NCT * NSTR * 128], F8,
                                        tag="wall_l", name="wall_l")

                def load_xt_chunk(tch, ctn0, ctn1, xt_t=None, parts=(0, 1)):
                    if xt_t is None:
                        xt_t = (xt_pool.tile([128, NCT * 512], F8, tag="xtch_h",
                                             name="xtch_h"),
                                xt_pool.tile([128, NCT * 512], F8, tag="xtch_l",
                                             name="xtch_l"))
                    srcs = ((xt_t[0], xT_hi), (xt_t[1], xT_lo))
                    for pi in parts:
                        xt_s, src_t = srcs[pi]
                        src = src_t[ctn0 * 128:ctn1 * 128,
                                    tch * 512:(tch + 1) * 512]
                        nc.sync.dma_start(
                            xt_s[:, ctn0 * 512:ctn1 * 512].rearrange(
                                "p (c t) -> p c t", t=512),
                            src.rearrange("(c p) t -> p c t", p=128))
                    return xt_t

                def load_w(c0, c1, w_all, wq):
                    nc.sync.dma_start(
                        w_all[:, c0 * NSTR * 128:c1 * NSTR * 128]
                        .rearrange("p (c t) -> p c t", t=NSTR * 128),
                        wq[c0 * 128:c1 * 128, :]
                        .rearrange("(c p) t -> p c t", p=128))

                # first chunk: small leading pieces so the first matmuls
                # start ~4us in; chunk 0 is emitted strip-pair-major with
                # term groups ordered so late-arriving pieces (w_lo, x_lo)
                # are consumed last
                load_w(0, 2, w_hi_all, wq_hi)
                xt_first = load_xt_chunk(0, 0, 2, parts=(0,))
                load_w(2, 4, w_hi_all, wq_hi)
                load_xt_chunk(0, 2, 4, xt_t=xt_first, parts=(0,))
                load_w(4, 10, w_hi_all, wq_hi)
                load_xt_chunk(0, 4, 10, xt_t=xt_first, parts=(0,))
                load_w(10, NCT, w_hi_all, wq_hi)
                load_xt_chunk(0, 10, NCT, xt_t=xt_first, parts=(0,))
                load_w(0, 8, w_lo_all, wq_lo)
                load_w(8, NCT, w_lo_all, wq_lo)
                load_xt_chunk(0, 0, 8, xt_t=xt_first, parts=(1,))
                load_xt_chunk(0, 8, NCT, xt_t=xt_first, parts=(1,))
                nc.sync.dma_start(
                    bq_sb[:, 0:NSTR].rearrange("p (c t) -> p c t", t=1),
                    bqkv[:, :].rearrange("(c p) t -> p c t", p=128))
                make_identity(nc, ident[:])

                # interleave schedule: after qkv for batch 0 lands (chunk 3),
                # its attention blocks weave into chunks 4..7 using the last
                # two qkv strips of each chunk as pipeline filler
                inter = {}
                if NTCH == 8 and NQC == 4:
                    inter = {4: [(0, 0, 0), (0, 0, 1)],
                             5: [(0, 0, 2), (0, 0, 3)],
                             6: [(1, 0, 0), (1, 0, 1)],
                             7: [(1, 0, 2), (1, 0, 3)]}

                for tch in range(NTCH):
                    if tch == 5:
                        load_wp()
                    if tch == min(1, NTCH - 1):
                        # constants land after the first xT burst is in flight
                        nc.sync.dma_start(ct_sb[:], ctil[:, :])
                        nc.sync.dma_start(st_sb[:], stil[:, :])
                        nc.sync.dma_start(cm_sb[:], cmask[:, :])
                        nc.sync.dma_start(bp_sb[:], bpb[:, :])
                    tw = (tch * 512) % T        # token offset within batch
                    tok = slice(tw, tw + 512)
                    if tch == 0:
                        xts = xt_first
                    else:
                        xts = load_xt_chunk(tch, 0, NCT)
                    # combined q/k staging tile so the RoPE pair-swap is two
                    # strided DMAs per chunk instead of two per strip
                    qkstage = stage_pool.tile([128, 4 * 512], F16, tag="qkst",
                                              name="qkst")

                    NPAIR = NCT // 2
                    NMM = 3 * NPAIR
                    terms = ((w_hi_all, xts[0]), (w_lo_all, xts[0]),
                             (w_hi_all, xts[1]))

                    def qkv_stage(s):
                        return qkstage[:, s * 512:(s + 1) * 512]

                    def emit_vdir(j):
                        # v strip computed directly in [token, d] layout
                        # (lhsT = x pair, rhs = w pair): no PE transpose.
                        # Pool copies psum -> v_sb with the 1/SW rescale;
                        # the v bias is folded into b_proj host-side.
                        s = 4 + j
                        for blk in range(4):
                            ps = vt_ps.tile([128, 128], F32, tag="vtp",
                                            name="vtp")
                            k = 0
                            for w_all, xt_s in terms:
                                w3 = w_all[:].rearrange("p (c x) -> p c x",
                                                        x=NSTR * 128)
                                x3 = xt_s[:].rearrange("p (c t) -> p c t",
                                                       t=512)
                                for c in range(NPAIR):
                                    nc.tensor.matmul(
                                        ps[:],
                                        x3[:, 2 * c:2 * c + 2,
                                           blk * 128:(blk + 1) * 128],
                                        w3[:, 2 * c:2 * c + 2,
                                           s * 128:(s + 1) * 128],
                                        start=(k == 0), stop=(k == NMM - 1),
                                        perf_mode=DR)
                                    k += 1
                            tt = tch * 4 + blk
                            nc.scalar.activation(
                                v_sb[:, j * BT + tt * 128:
                                     j * BT + (tt + 1) * 128],
                                ps[:], mybir.ActivationFunctionType.Identity,
                                scale=1.0 / SW)

                    def emit_strip(s):
                        ps = mm_ps.tile([128, 512], F32, tag="ps512",
                                        name="qkvps")
                        k = 0
                        for w_all, xt_s in terms:
                            w3 = w_all[:].rearrange("p (c x) -> p c x",
                                                    x=NSTR * 128)
                            x3 = xt_s[:].rearrange("p (c t) -> p c t", t=512)
                            for c in range(NPAIR):
                                nc.tensor.matmul(
                                    ps[:],
                                    w3[:, 2 * c:2 * c + 2,
                                       s * 128:(s + 1) * 128],
                                    x3[:, 2 * c:2 * c + 2, :],
                                    start=(k == 0), stop=(k == NMM - 1),
                                    perf_mode=DR)
                                k += 1
                        st_t = qkv_stage(s)
                        nc.scalar.activation(
                            st_t[:], ps[:], mybir.ActivationFunctionType.Identity,
                            bias=bq_sb[:, s:s + 1], scale=1.0 / SW)

                    units = inter.get(tch, [])
                    if tch == 0:
                        # term-major across the 4 q/k strips at once (4 psum
                        # banks: 3 mm_ps + 1 y_ps, all idle before attention
                        # starts). Term 0 streams against the w_hi/x_hi
                        # loads; w_lo and x_lo land before terms 1 and 2
                        # consume them.
                        pa4 = ([mm_ps.tile([128, 512], F32, tag="ps512",
                                           name="qkvps") for _ in range(3)] +
                               [y_ps.tile([128, 512], F32, tag="psy",
                                          name="qkvpsy")])
                        k = 0
                        for w_all, xt_s in terms:
                            w3 = w_all[:].rearrange("p (c x) -> p c x",
                                                    x=NSTR * 128)
                            x3 = xt_s[:].rearrange("p (c t) -> p c t", t=512)
                            for c in range(NPAIR):
                                for s in range(4):
                                    nc.tensor.matmul(
                                        pa4[s][:],
                                        w3[:, 2 * c:2 * c + 2,
                                           s * 128:(s + 1) * 128],
                                        x3[:, 2 * c:2 * c + 2, :],
                                        start=(k == 0), stop=(k == NMM - 1),
                                        perf_mode=DR)
                                k += 1
                        for s in range(4):
                            nc.scalar.activation(
                                qkv_stage(s), pa4[s][:],
                                mybir.ActivationFunctionType.Identity,
                                bias=bq_sb[:, s:s + 1], scale=1.0 / SW)
                        emit_vdir(0)
                        emit_vdir(1)
                    else:
                        for s in range(4):
                            emit_strip(s)
                        if units:
                            attn_qc(*units[0], filler=[lambda: emit_vdir(0)])
                            attn_qc(*units[1], filler=[lambda: emit_vdir(1)])
                        else:
                            emit_vdir(0)
                            emit_vdir(1)
                    # pair-swap of the whole q/k staging block (2 strided DMAs)
                    sw = rope_pool.tile([128, 4 * 512], F16, tag="swqk",
                                        name="swqk")
                    nc.sync.dma_start(sw[0:127:2, :], qkstage[1:128:2, :])
                    nc.sync.dma_start(sw[1:128:2, :], qkstage[0:127:2, :])
                    for j in range(HL):
                        # RoPE on q and k for this (j, tch)
                        for part, dst in ((j, qr), (2 + j, kr)):
                            st_t = qkstage[:, part * 512:(part + 1) * 512]
                            sw_t = sw[:, part * 512:(part + 1) * 512]
                            dstsl = dst[:, j * BT + tch * 512: j * BT + (tch + 1) * 512]
                            tmp = rope_pool.tile([128, 512], F16, tag=f"rt{j}",
                                                 name=f"rt{j}")
                            nc.vector.tensor_mul(dstsl, st_t, ct_sb[:, tok])
                            nc.vector.tensor_mul(tmp[:], sw_t, st_sb[:, tok])
                            nc.vector.tensor_add(dstsl, dstsl, tmp[:])

            # ============ Phase B: remaining attention + projection =========
            with (
                tc.tile_pool(name="ygs", bufs=6) as ygs_pool,
                tc.tile_pool(name="part", bufs=1) as part_pool,
                tc.tile_pool(name="ot", bufs=6) as ot_pool,
                tc.tile_pool(name="o_ps", bufs=1, space="PSUM") as o_ps,
            ):
                def wp_pair(wa, par, m):
                    # adjacent parity-major blocks (8*par+2m, 8*par+2m+1)
                    off = (8 * par + 2 * m) * OC
                    return wa[:, off:off + 2 * OC].rearrange(
                        "p (two n) -> p two n", two=2)

                # mid-stream projection groups cycle 3 psum banks (op0,
                # op1, plus the l_ps bank freed by the matmul-free softmax
                # denominator); after attention drains, tail groups add the
                # psy bank for a 4-deep rotation
                cyc3 = [
                    lambda: o_ps.tile([128, OC], F32, tag="op0", name="op0"),
                    lambda: o_ps.tile([128, OC], F32, tag="op1", name="op1"),
                    lambda: l_ps.tile([128, OC], F32, tag="lps", name="lpsp"),
                ]
                cyc4 = cyc3 + [
                    lambda: y_ps.tile([128, OC], F32, tag="psy", name="psyp"),
                ]
                deep_idx = [0]
                mid_idx = [0]

                def prefetch_yg(b, par, tg0, halves=1):
                    """Issue the merged agout->SBUF loads (fp8 hi + lo) for
                    one projection group ahead of its compute (optionally in
                    column halves to cut first-use latency)."""
                    ng = min(4, NTT - tg0)
                    h = tg0 * 128 // hw_
                    ccdep = cc_insts[(par, b)][h]
                    c0 = tg0 * 128 - h * hw_
                    step = ng * 128 // halves
                    ys = []
                    for hl, base in enumerate((c0, hw_ + c0)):
                        yg = ygs_pool.tile([128, 8 * ng * 128], F8,
                                           tag=f"ygall{hl}",
                                           name=f"ygall{hl}")
                        for hh in range(halves):
                            src_ap = agout[(par, b, h)][
                                :, base + hh * step:base + (hh + 1) * step]
                            dst = yg[:].rearrange(
                                "p (blk t) -> p blk t",
                                t=ng * 128)[:, :, hh * step:(hh + 1) * step]
                            d = nc.sync.dma_start(
                                dst, src_ap.rearrange("(blk p) t -> p blk t",
                                                      p=128))
                            di = d.ins if hasattr(d, "ins") else d
                            add_dep_helper(di, ccdep,
                                           reason="proj reads agout after collective")
                        ys.append(yg)
                    return tuple(ys)

                deferred_stores = []

                def flush_stores():
                    for ot_t, r0, npair in deferred_stores:
                        nc.sync.dma_start(
                            out[r0:r0 + npair * 128, :].rearrange(
                                "(blk p) t -> p blk t", p=128),
                            ot_t[:].rearrange("p (blk t) -> p blk t", t=OC))
                    deferred_stores.clear()

                def proj_group(b, par, tg0, first, deep=False, yg_pre=None,
                               defer_store=False, p0s=None):
                    """One 4-token-tile projection unit for heads of parity
                    `par` (all in one agout buffer): 1 merged yg DMA + 2 psum
                    pair-groups.

                    first=True: psum + bias -> fp16 partial tiles in SBUF.
                    first=False: psum + partial -> merged output-store DMA.
                    """
                    ng = min(4, NTT - tg0)
                    ygh, ygl = yg_pre if yg_pre is not None else \
                        prefetch_yg(b, par, tg0)

                    def y3(ya):
                        return ya[:].rearrange("p (blk t) -> p blk t",
                                               t=ng * 128)

                    def alloc_ps():
                        if deep:
                            t = cyc4[deep_idx[0] % 4]()
                            deep_idx[0] += 1
                        else:
                            t = cyc3[mid_idx[0] % 3]()
                            mid_idx[0] += 1
                        return t

                    for p0 in (range(0, ng, 2) if p0s is None else p0s):
                        npair = min(2, ng - p0)
                        pss = [alloc_ps() for _ in range(npair)]
                        k = 0
                        for ya, wa in ((ygh, wp_hi_all), (ygl, wp_hi_all),
                                       (ygh, wp_lo_all)):
                            yv = y3(ya)
                            for m in range(4):
                                for i in range(npair):
                                    nc.tensor.matmul(
                                        pss[i][:],
                                        yv[:, 2 * m:2 * m + 2,
                                           (p0 + i) * 128:(p0 + i + 1) * 128],
                                        wp_pair(wa, par, m),
                                        start=(k == 0), stop=(k == 11),
                                        perf_mode=DR)
                                k += 1
                        if first:
                            for i in range(npair):
                                tt = tg0 + p0 + i
                                pt_t = part_pool.tile([128, OC], F16,
                                                      tag=f"part{b}_{tt}",
                                                      name=f"part{b}_{tt}")
                                nc.vector.tensor_add(pt_t[:], pss[i][:], bp_sb[:])
                                partial[(b, tt)] = pt_t
                        else:
                            # pair of token tiles -> one [128, 2*OC] SBUF
                            # tile -> one merged output-store DMA
                            ot = ot_pool.tile([128, npair * OC], F16,
                                              tag="ot", name="ot")
                            for i in range(npair):
                                tt = tg0 + p0 + i
                                nc.vector.tensor_add(
                                    ot[:, i * OC:(i + 1) * OC], pss[i][:],
                                    partial[(b, tt)][:])
                            r0 = b * T + (tg0 + p0) * 128
                            nc.sync.dma_start(
                                out[r0:r0 + npair * 128, :].rearrange(
                                    "(blk p) t -> p blk t", p=128),
                                ot[:].rearrange("p (blk t) -> p blk t",
                                                t=OC))

                # Emission order: batch-1 attention blocks carry projection
                # groups as filler (even-parity partials during (0,1), odd
                # batch-0 finals + even batch-1 partials during (1,1)),
                # leaving only the last parity-1 groups after the final AG.
                def tg_of(qc):
                    return 8 * (qc // 2) + 4 * (qc % 2)

                # fallback: any batch-0 units not interleaved into the chunks
                done_b0 = {u for us in inter.values() for u in us}
                for j in range(HL):
                    for qc in range(NQC):
                        if (j, 0, qc) not in done_b0:
                            attn_qc(j, 0, qc)

                for qc in range(NQC):
                    attn_qc(0, 1, qc, filler=[functools.partial(
                        proj_group, 0, 0, tg_of(qc), True)])
                pre = {}
                for qc in range(NQC):
                    if qc == NQC - 1:
                        # prefetch the already-gatherable tail loads before
                        # the last attention unit so no store DMA can
                        # head-of-line block them on the SP queue
                        pre[(0, 1, 12)] = prefetch_yg(0, 1, 12)
                        pre[(1, 1, 4)] = prefetch_yg(1, 1, 4)
                    attn_qc(1, 1, qc, filler=[functools.partial(
                        proj_group, 1, 0, tg_of(qc), True)],
                        last=(qc == NQC - 1))
                    if qc == NQC - 1:
                        # the final AG quarter is now in flight: issue its
                        # loads in column halves to cut first-use latency
                        pre[(1, 1, 8)] = prefetch_yg(1, 1, 8, halves=2)
                        pre[(1, 1, 12)] = prefetch_yg(1, 1, 12, halves=2)
                    if qc < NQC - 1:
                        proj_group(0, 1, tg_of(qc), first=False)
                    if qc >= 2:
                        proj_group(1, 1, tg_of(qc) - 8, first=False,
                                   yg_pre=pre.pop((1, 1, tg_of(qc) - 8), None))
                for tg0, dp in (((0, 1, 12), False), ((1, 1, 8), True),
                                ((1, 1, 12), True)):
                    b_, par_, tg_ = tg0
                    proj_group(b_, par_, tg_, first=False, deep=dp,
                               yg_pre=pre.pop(tg0, None))
    nc.compile()
    return nc


def _split8(a):
    """fp8 e4m3 hi + residual lo decomposition of a float32 array."""
    import ml_dtypes
    hi = a.astype(ml_dtypes.float8_e4m3)
    lo = (a - hi.astype(np.float32)).astype(ml_dtypes.float8_e4m3)
    return hi, lo


def _prep_inputs(x, W_attn, b_attn, W_proj, b_proj, cos, sin, core, B, T):
    """Host-side shard prep for one core."""
    BT = B * T
    xT = np.ascontiguousarray(x.reshape(BT, C).T)
    xT_hi, xT_lo = _split8(xT)

    cols = []
    bvals = []
    for part in range(3):  # q, k, v
        for j in range(HL):
            h = 2 * core + j
            sl = slice(part * C + h * HD, part * C + (h + 1) * HD)
            cols.append(W_attn[:, sl])
            bvals.append(b_attn[sl])
    wq_hi, wq_lo = _split8(SW * np.concatenate(cols, axis=1))
    bqkv = np.concatenate(bvals).astype(np.float32).reshape(-1, 1)

    # RoPE tables: ctil[p, t] = cos[t, p//2]; stil[2i] = -sin, stil[2i+1] = +sin
    cosr = np.repeat(cos.T, 2, axis=0)  # [128, T]
    sinr = np.repeat(sin.T, 2, axis=0)
    sgn = np.where((np.arange(128) % 2) == 0, -1.0, 1.0)[:, None]
    ctil = cosr.astype(np.float16)
    stil = (sinr * sgn).astype(np.float16)

    # W_proj rows parity-major: even global heads' 128-row blocks first,
    # then odd heads' (matches the kernel's per-parity agout layout)
    worder = [g for g in range(H) if g % 2 == 0] + \
             [g for g in range(H) if g % 2 == 1]
    wp_c = np.concatenate(
        [W_proj[g * HD:(g + 1) * HD, core * OC:(core + 1) * OC]
         for g in worder], axis=0)
    wp_hi, wp_lo = _split8(SW * wp_c)
    # v bias passes through attention unchanged (sum_k P = 1): fold it into
    # the projection bias so the v strips need no bias add on-device
    b_v = b_attn[2 * C:3 * C].astype(np.float32)
    bp_eff = b_proj[core * OC:(core + 1) * OC].astype(np.float32) + \
        b_v @ W_proj[:, core * OC:(core + 1) * OC].astype(np.float32)
    bpb = np.tile(SW * bp_eff, (128, 1))
    ii, jj = np.mgrid[0:128, 0:128]
    # transposed causal triangle for the S^T orientation: row=k, col=q,
    # masked (additive -900 pre-scale) where k > q
    cmask = np.where(ii <= jj, 0.0, MASK_VAL).astype(np.float16)
    return {
        "xT_hi": xT_hi, "xT_lo": xT_lo, "wq_hi": wq_hi, "wq_lo": wq_lo,
        "bqkv": bqkv, "ctil": ctil, "stil": stil,
        "wp_hi": wp_hi, "wp_lo": wp_lo, "bpb": bpb, "cmask": cmask,
    }


@functools.lru_cache(maxsize=2)
def _built(B, T):
    return build(B=B, T=T)


_warmed = set()


def kernel(x, W_attn, b_attn, W_proj, b_proj, cos, sin):
    x = np.asarray(x, dtype=np.float32)
    W_attn = np.asarray(W_attn, dtype=np.float32)
    b_attn = np.asarray(b_attn, dtype=np.float32)
    W_proj = np.asarray(W_proj, dtype=np.float32)
    b_proj = np.asarray(b_proj, dtype=np.float32)
    cos = np.asarray(cos, dtype=np.float32)
    sin = np.asarray(sin, dtype=np.float32)

    B, T, Cv = x.shape
    assert Cv == C
    nc = _built(B, T)
    in_maps = [_prep_inputs(x, W_attn, b_attn, W_proj, b_proj, cos, sin, c, B, T)
               for c in range(N_CORES)]
    if (B, T) not in _warmed:
        # The very first execution of a freshly loaded NEFF has been observed
        # to deliver stale/uninitialized collective buffers; run once and
        # discard, then run for real.
        run_bass_kernel_spmd(nc, in_maps, core_ids=list(range(N_CORES)))
        _warmed.add((B, T))
    res = run_bass_kernel_spmd(nc, in_maps, core_ids=list(range(N_CORES)))
    outs = [np.asarray(res.results[c]["out"], dtype=np.float32)
            for c in range(N_CORES)]
    full = np.concatenate(outs, axis=1) * np.float32(1.0 / SW)  # [BT, C]
    return full.reshape(B, T, C).astype(np.float32)

